# revision 10
# baseline (speedup 1.0000x reference)
"""Depth-upsample module kernel for 8 TRN2 NeuronCores.

Pipeline per core (1/8 of batch*height), all-bf16 PE/DVE path:
  conv1 3x3 8->8 + bias + relu   (PE bf16 banded-dy matmuls; ACT relu)
  conv2 1x1 8->36 (raw)          (PE bf16, 2 matmuls per subpixel ab)
  E = exp(0.25*conv2 + 0.25*b2)  (ACT, PSUM -> EP[:, 0:W] bf16)
  P = E * unfolded-depth         (DVE bf16 2x, -> EP[:, W:2W])
  Den|Num = band matmuls over EP (PE, one [128, 2W] 3-bank PSUM tile)
  RD = 1/Den                     (ACT Reciprocal, PSUM -> SBUF bf16)
  NS = Num                       (DVE copy, PSUM -> SBUF bf16)
  O[r, 2x+b] = NS*RD             (DVE bf16 2x muls, strided-column writes)
  store O rows (contiguous 2.5KB descriptors), bf16; host upcasts to f32.

DMA rings: xh loads + output stores on sync (HWDGE/SP), unfolded depth on
gpsimd (SWDGE), consts on scalar (HWDGE/ACT, once).

Layout: row-blocks of R=14 output rows; SBUF partitions pack (row, channel):
  conv input  xb   [(r16,i8)=128, 642] bf16
  conv1 out   Y    [(r14,o8)=112, 640] bf16
  EP               [(r14,k9)=126, 1280] bf16 per ab: E | P
  psumND           [128, 1280] f32: quadrant ab rows 32ab..+R, Den | Num
  out interleave   O_a [14, 1280] bf16; O_a[r, 2x+b] = result for ab=2a+b
"""

import numpy as np
import ml_dtypes

H, W = 512, 640
N_IMG, C_IN = 4, 8
HALF = H // 2           # rows per core (shard = image x half)
RB = 14                 # output rows per block
WP = W + 2              # padded width
CWB = 854               # bf16 const cols: w1(336) w2(504) band(14)


def _build_consts(conv1_w, conv1_b, conv2_w, conv2_b):
    f32 = np.float32
    # lhsT1[dx, (r,i), (r',o)] = W1[o,i,r-r',dx] for r-r' in {0,1,2}
    lhsT1 = np.zeros((3, 128, 112), f32)
    for dx in range(3):
        for rp in range(14):
            for dy in range(3):
                r = rp + dy
                lhsT1[dx, r * 8:(r + 1) * 8, rp * 8:(rp + 1) * 8] = \
                    conv1_w[:, :, dy, dx].T  # [i, o]
    # lhsT2[ab, (r,i), (r,k)] = W2[4k+ab, i]
    lhsT2 = np.zeros((4, 112, 126), f32)
    w2 = conv2_w[:, :, 0, 0]  # [36, 8]
    for ab in range(4):
        for r in range(14):
            for k in range(9):
                lhsT2[ab, r * 8:(r + 1) * 8, r * 9 + k] = w2[k * 4 + ab, :]
    # band[(r,k), r'] = 1 iff r == r'
    band = np.zeros((126, 14), f32)
    for r in range(14):
        band[r * 9:(r + 1) * 9, r] = 1
    b1v = np.tile(conv1_b.astype(f32), 14)[:, None]            # [112,1]
    b2v = np.zeros((4, 126, 1), f32)
    for ab in range(4):
        for r in range(14):
            for k in range(9):
                b2v[ab, r * 9 + k, 0] = 0.25 * float(conv2_b[k * 4 + ab])
    return lhsT1, lhsT2, band, b1v, b2v


def _pack_consts(lhsT1, lhsT2, band, b1v, b2v):
    cb = np.zeros((128, CWB), ml_dtypes.bfloat16)
    for dx in range(3):
        cb[:, 112 * dx: 112 * (dx + 1)] = lhsT1[dx]
    for ab in range(4):
        cb[:112, 336 + 126 * ab: 336 + 126 * (ab + 1)] = lhsT2[ab]
    cb[:126, 840:854] = band
    cf = np.zeros((128, 5), np.float32)
    cf[:112, 0:1] = b1v
    for ab in range(4):
        cf[:126, 1 + ab: 2 + ab] = b2v[ab]
    return cb, cf


def _build_bass():
    import concourse.bass as bass
    import concourse.bacc as bacc
    import concourse.tile as tile
    from concourse import mybir

    f32 = mybir.dt.float32
    bf16 = mybir.dt.bfloat16
    nc = bacc.Bacc(None, target_bir_lowering=False)

    X = nc.dram_tensor("xh", [C_IN, HALF + 2, WP], bf16, kind="ExternalInput")
    DUNF = nc.dram_tensor("dunf", [HALF * 9, W], bf16, kind="ExternalInput")
    CB = nc.dram_tensor("cb", [128, CWB], bf16, kind="ExternalInput")
    CF = nc.dram_tensor("cf", [128, 5], f32, kind="ExternalInput")
    OUT = nc.dram_tensor("out", [2, 2 * HALF, W], bf16,
                         kind="ExternalOutput")

    nblocks = (HALF + RB - 1) // RB  # 19 (last block R=4)

    with tile.TileContext(nc) as tc:
        with (
            tc.tile_pool(name="consts", bufs=1) as consts,
            tc.tile_pool(name="xp", bufs=3) as xp,
            tc.tile_pool(name="dp", bufs=3) as dp,
            tc.tile_pool(name="yp", bufs=2) as yp,
            tc.tile_pool(name="ep", bufs=6) as ep,
            tc.tile_pool(name="rp", bufs=2) as rp,
            tc.tile_pool(name="np_", bufs=2) as nsp,
            tc.tile_pool(name="op", bufs=4) as op,
            tc.tile_pool(name="psA", bufs=2, space="PSUM") as psA,
            tc.tile_pool(name="psnd", bufs=1, space="PSUM") as psnd,
        ):
            cb = consts.tile([128, CWB], bf16, tag="cb")
            nc.scalar.dma_start(out=cb, in_=CB[:])
            cf = consts.tile([128, 5], f32, tag="cf")
            nc.scalar.dma_start(out=cf, in_=CF[:])
            w1t = [cb[:, 112 * dx: 112 * (dx + 1)] for dx in range(3)]
            w2t = [cb[:112, 336 + 126 * ab: 336 + 126 * (ab + 1)]
                   for ab in range(4)]
            bandt = cb[:126, 840:854]
            b1t = cf[:112, 0:1]
            b2t = [cf[:126, 1 + ab: 2 + ab] for ab in range(4)]

            for b in range(nblocks):
                R = min(RB, HALF - RB * b)
                Rin = R + 2
                s = RB * b
                kp = R * 9   # partitions in (r,k) tiles
                yq = R * 8   # partitions in (r,o) tiles

                # --- load conv input block [(r,i), w] bf16, sync ring ---
                xb = xp.tile([128, WP], bf16, tag="xb")
                x_in = bass.AP(
                    tensor=X[:].tensor, offset=s * WP,
                    ap=[[WP, Rin], [(HALF + 2) * WP, C_IN], [1, WP]],
                )
                nc.sync.dma_start(out=xb[: Rin * 8], in_=x_in)

                # --- load unfolded depth [(r,k), x] bf16, gpsimd ring ---
                dunf = dp.tile([126, W], bf16, tag="dunf")
                nc.gpsimd.dma_start(out=dunf[:kp], in_=DUNF[9 * s: 9 * s + kp])

                # --- conv1: 3 dx matmuls x 2 col chunks -> psum ---
                psum1 = psA.tile([128, W], f32, tag="mm")
                for c0, cn in ((0, 512), (512, 128)):
                    for dx in range(3):
                        nc.tensor.matmul(
                            psum1[:yq, c0:c0 + cn],
                            w1t[dx][: Rin * 8, :yq],
                            xb[: Rin * 8, dx + c0: dx + c0 + cn],
                            start=(dx == 0), stop=(dx == 2),
                        )

                # --- bias+relu -> Y (SBUF bf16) ---
                Y = yp.tile([112, W], bf16, tag="y")
                nc.scalar.activation(
                    out=Y[:yq], in_=psum1[:yq],
                    func=mybir.ActivationFunctionType.Relu,
                    bias=b1t[:yq], scale=1.0,
                )

                # --- conv2 + exp per ab; EP = [E | E*depth] ---
                psumND = psnd.tile([128, 2 * W], f32, tag="psumnd")
                for ab in range(4):
                    psum2 = psA.tile([128, W], f32, tag="mm")
                    for c0, cn in ((0, 512), (512, 128)):
                        nc.tensor.matmul(
                            psum2[:kp, c0:c0 + cn],
                            w2t[ab][:yq, :kp],
                            Y[:yq, c0:c0 + cn],
                            start=True, stop=True,
                        )
                    EP = ep.tile([126, 2 * W], bf16, tag="ep")
                    nc.scalar.activation(
                        out=EP[:kp, 0:W], in_=psum2[:kp],
                        func=mybir.ActivationFunctionType.Exp,
                        bias=b2t[ab][:kp], scale=0.25,
                    )
                    nc.vector.tensor_mul(
                        EP[:kp, W:2 * W], EP[:kp, 0:W], dunf[:kp])
                    # tap reduction: Den | Num into psum quadrant ab
                    for c0, cn in ((0, 512), (512, 512), (1024, 256)):
                        nc.tensor.matmul(
                            psumND[32 * ab: 32 * ab + R, c0:c0 + cn],
                            bandt[:kp, :R], EP[:kp, c0:c0 + cn],
                            start=True, stop=True,
                            tile_position=(0, 32 * ab),
                        )

                # --- divide + column interleave into [R, 2W] row tiles ---
                RDf = rp.tile([128, W], f32, tag="rdf")
                nc.vector.reciprocal_approx_fast(out=RDf, in_=psumND[:, 0:W])
                RD = rp.tile([128, W], bf16, tag="rd")
                nc.vector.tensor_copy(RD, RDf)
                NS = nsp.tile([128, W], bf16, tag="ns")
                nc.vector.tensor_copy(NS, psumND[:, W:2 * W])
                O = op.tile([128, W], bf16, tag="o")
                for a in range(2):
                    for bb in range(2):
                        ab = 2 * a + bb
                        p0 = 32 * (2 * bb + a)
                        nc.vector.tensor_mul(
                            O[p0:p0 + R],
                            NS[32 * ab: 32 * ab + R],
                            RD[32 * ab: 32 * ab + R],
                        )
                # stores: O[32(2bb+a)+r] -> OUT[bb, 2(s+r)+a, :]
                for bb in range(2):
                    eng = nc.sync if bb == 0 else nc.gpsimd
                    for a in range(2):
                        p0 = 32 * (2 * bb + a)
                        o_out = bass.AP(
                            tensor=OUT[:].tensor,
                            offset=(bb * 2 * HALF + 2 * s + a) * W,
                            ap=[[2 * W, R], [1, W]],
                        )
                        eng.dma_start(out=o_out, in_=O[p0:p0 + R])

    nc.compile()
    return nc


_NC_CACHE = None


def prep_inputs(depth, cost_volume, conv1_w, conv1_b, conv2_w, conv2_b):
    bf = ml_dtypes.bfloat16
    depth = np.asarray(depth, np.float32)
    cv = np.asarray(cost_volume, np.float32).reshape(N_IMG, C_IN, H, W)
    lhsT1, lhsT2, band, b1v, b2v = _build_consts(
        np.asarray(conv1_w, np.float32), np.asarray(conv1_b, np.float32),
        np.asarray(conv2_w, np.float32), np.asarray(conv2_b, np.float32))
    cb, cf = _pack_consts(lhsT1, lhsT2, band, b1v, b2v)

    # halo'd, zero-padded shards: core c = 2*n + h
    sw = np.lib.stride_tricks.sliding_window_view
    in_maps = []
    for n in range(N_IMG):
        cvp = np.zeros((C_IN, H + 2, WP), bf)
        cvp[:, 1:H + 1, 1:W + 1] = cv[n]
        dpad = np.zeros((H + 2, WP), np.float32)
        dpad[1:H + 1, 1:W + 1] = depth[n]
        # unfold: du[(r*9 + ky*3 + kx), x] = dpad[r+ky, x+kx]
        win = sw(dpad, (3, W + 2))[:H, 0]                # [H,3,W+2]
        du = np.stack([win[:, :, kx:kx + W] for kx in range(3)], 2)
        du = du.reshape(H * 9, W).astype(bf)
        for h in range(2):
            r0 = h * HALF
            in_maps.append({
                "xh": np.ascontiguousarray(cvp[:, r0:r0 + HALF + 2, :]),
                "dunf": np.ascontiguousarray(du[9 * r0: 9 * (r0 + HALF)]),
                "cb": cb,
                "cf": cf,
            })
    return in_maps


def kernel(depth, cost_volume, conv1_w, conv1_b, conv2_w, conv2_b):
    global _NC_CACHE
    from concourse.bass_utils import run_bass_kernel_spmd

    in_maps = prep_inputs(depth, cost_volume, conv1_w, conv1_b,
                          conv2_w, conv2_b)
    if _NC_CACHE is None:
        _NC_CACHE = _build_bass()
    res = run_bass_kernel_spmd(_NC_CACHE, in_maps, core_ids=list(range(8)))
    out = np.empty((N_IMG, 2 * H, 2 * W), np.float32)
    for c, r in enumerate(res.results):
        n, h = c // 2, c % 2
        v = out[n, 2 * h * HALF: 2 * (h + 1) * HALF]
        v[:, 0::2] = r["out"][0]
        v[:, 1::2] = r["out"][1]
    return out
